# revision 10
# baseline (speedup 1.0000x reference)
"""MemN2N (3-hop memory network) forward pass on 8 Trainium2 NeuronCores.

Optimized v2. Strategy (per spec sharding hint):
 - Data-parallel over batch (4/core) for gathers + hops; vocab-sharded head
   (4000/core) with AllGather of u3 and of the per-batch exp-sums.
 - Embedding tables fused into one [V, 4E] bf16 table; position encoding is
   rank-2 (1 + a_l b_e) so the word-sum reduces to two weighted sums done on
   the TensorEngine with a block-diagonal weight matrix.
 - Hops: scores for all 4 local batches in ONE accumulated matmul group
   [4, 400] (cross-products masked off via host-precomputed mask tensors),
   softmax on 4 partitions, p broadcast+block-diag-extract via a ones-matmul,
   o via one big tensor_tensor + reduce. All score/broadcast matmuls run in
   float32r (full PE rate at >=256 cols).
 - Head: wx matmuls bf16; BatchNorm rstd via bit-trick+Newton rsqrt on DVE
   (avoids Sqrt act-table swaps); log-softmax with log(VM) folded in on host
   (pad vocab rows = -1e30 so exp-sums need no extra memsets).
 - The 2.5MB wt/logvm const loads are dependency-gated behind the LAST
   story-gather chunk's flush so they run during the hop phase and stay
   entirely out of the gather-critical DMA window.

kernel(**inputs) takes FULL unsharded inputs, returns FULL (32, 32000) f32.
"""
import sys
sys.path.insert(0, '/opt/trn_rl_repo')
import numpy as np
import ml_dtypes
from contextlib import ExitStack

import concourse.bass as bass
import concourse.bacc as bacc
import concourse.tile as tile
from concourse import mybir
from concourse.alu_op_type import AluOpType
from concourse.bass_utils import run_bass_kernel_spmd

F32 = mybir.dt.float32
F32R = mybir.dt.float32r
BF16 = mybir.dt.bfloat16
I16 = mybir.dt.int16
AF = mybir.ActivationFunctionType
AX = mybir.AxisListType

B, M, L, LQ, E, V = 32, 100, 30, 30, 256, 32000
NC_ = 8
BL = B // NC_          # 4 local batches
VL = V // NC_          # 4000 local vocab
CH = 128               # head vocab-chunk size
NCH = 32               # chunks (last chunk only 32 rows)
NQ = 8                 # gather chunk: 8 m-columns -> 1024 indices
EH = E // 2            # 128

_cache = {}


def _consts():
    a = (np.arange(1, L + 1, dtype=np.float64) - (L + 1) / 2.0)
    b = 4.0 * (np.arange(1, E + 1, dtype=np.float64) - (E + 1) / 2.0) / (E * L)
    wmat = np.zeros((128, 8), np.float32)
    for r in range(4):
        for l in range(L):
            wmat[32 * r + l, 2 * r + 0] = 1.0
            wmat[32 * r + l, 2 * r + 1] = a[l]
    bvec = b.astype(np.float32).reshape(2, EH).T.copy()               # [128,2]
    ones1 = np.ones((1, 128), np.float32)
    on4 = np.ones((4, 128), np.float32)
    onc = np.ones((128, 1), np.float32)
    ident = np.eye(128, dtype=np.float32)
    return wmat, bvec, ones1, on4, onc, ident


def _wrap_idx(flat):
    n = flat.shape[0]
    wr = flat.reshape(n // 16, 16).T.astype(np.int16)
    return np.tile(wr, (8, 1)).copy()


def _r(ap_):
    """bitcast an AP to float32r for full-rate PE"""
    return ap_.bitcast(F32R)


def build_nc(repeat=1, stage=5, nhop=3, xcoll=0, serial=0, f32r=1, dbg_terms=0, chain=0):
    FR = F32R if f32r else F32
    nc = bacc.Bacc("TRN2", target_bir_lowering=False, debug=False,
                   num_devices=NC_, dynamic_dma_scratch_size=65536)

    # ---- DRAM I/O ----
    tabc = nc.dram_tensor("tabc", [V, 4 * E], BF16, kind="ExternalInput").ap()
    sidx = nc.dram_tensor("sidx", [128, M * 128 // 16], I16, kind="ExternalInput").ap()
    qidx = nc.dram_tensor("qidx", [128, 8], I16, kind="ExternalInput").ap()
    qmc = nc.dram_tensor("qmc", [128, 1], F32, kind="ExternalInput").ap()
    mmi = nc.dram_tensor("mmi", [4, 400], F32, kind="ExternalInput").ap()
    mai = nc.dram_tensor("mai", [4, 400], F32, kind="ExternalInput").ap()
    wti = nc.dram_tensor("wti", [2, 128, VL], BF16, kind="ExternalInput").ap()
    gbi = nc.dram_tensor("gbi", [128, 2, NCH], F32, kind="ExternalInput").ap()
    vmi = nc.dram_tensor("vmi", [128, NCH, B], F32, kind="ExternalInput").ap()
    wmi = nc.dram_tensor("wmi", [128, 8], BF16, kind="ExternalInput").ap()
    bvi = nc.dram_tensor("bvi", [128, 2], F32, kind="ExternalInput").ap()
    on1 = nc.dram_tensor("on1", [1, 128], F32, kind="ExternalInput").ap()
    on4i = nc.dram_tensor("on4i", [4, 128], FR, kind="ExternalInput").ap()
    onci = nc.dram_tensor("onci", [128, 1], F32, kind="ExternalInput").ap()
    idi = nc.dram_tensor("idi", [128, 128], F32, kind="ExternalInput").ap()
    out = nc.dram_tensor("out", [128, NCH, B], F32, kind="ExternalOutput").ap()

    u3_loc = nc.dram_tensor("u3_loc", [2, 128, BL], F32).ap()
    u3_gth = nc.dram_tensor("u3_gth", [NC_, 2, 128, BL], F32,
                            addr_space="Shared").ap()
    lse_loc = nc.dram_tensor("lse_loc", [B], F32).ap()
    lse_gth = nc.dram_tensor("lse_gth", [NC_, B], F32,
                             addr_space="Shared").ap()

    with tile.TileContext(nc) as tc, ExitStack() as ctx:
        cons = ctx.enter_context(tc.tile_pool(name="cons", bufs=1))
        embp = ctx.enter_context(tc.tile_pool(name="embp", bufs=1))
        rt_p = ctx.enter_context(tc.tile_pool(name="rt", bufs=2))
        tmp = ctx.enter_context(tc.tile_pool(name="tmp", bufs=4))
        big = ctx.enter_context(tc.tile_pool(name="big", bufs=1))
        up = ctx.enter_context(tc.tile_pool(name="up", bufs=2))
        pp_e = ctx.enter_context(tc.tile_pool(name="pp_e", bufs=(1 if dbg_terms else 2), space="PSUM"))
        pp_s = ctx.enter_context(tc.tile_pool(name="pp_s", bufs=1, space="PSUM"))
        pp_b = ctx.enter_context(tc.tile_pool(name="pp_b", bufs=1, space="PSUM"))
        pp_w = ctx.enter_context(tc.tile_pool(name="pp_w", bufs=1, space="PSUM"))
        pp_t = ctx.enter_context(tc.tile_pool(name="pp_t", bufs=(1 if dbg_terms else 2), space="PSUM"))

        # ---- early consts (sync queue: gather-critical) ----
        sidx_sb = cons.tile([128, M * 8], I16)
        nc.sync.dma_start(sidx_sb[:], sidx)
        qidx_sb = cons.tile([128, 8], I16)
        nc.sync.dma_start(qidx_sb[:], qidx)
        wmat = cons.tile([128, 8], BF16)
        nc.sync.dma_start(wmat[:], wmi)
        bvec = cons.tile([128, 2], F32)
        nc.sync.dma_start(bvec[:], bvi)
        ones1 = cons.tile([1, 128], F32)
        nc.sync.dma_start(ones1[:], on1)
        on4 = cons.tile([4, 128], FR)
        nc.sync.dma_start(on4[:], on4i)
        onc = cons.tile([128, 1], F32)
        nc.sync.dma_start(onc[:], onci)
        qm_sb = cons.tile([128, 1], F32)
        nc.sync.dma_start(qm_sb[:], qmc)
        mm_sb = cons.tile([4, 400], F32)
        nc.sync.dma_start(mm_sb[:], mmi)
        ma_sb = cons.tile([4, 400], F32)
        nc.sync.dma_start(ma_sb[:], mai)
        eps_t = cons.tile([128, 1], F32)
        nc.vector.memset(eps_t[:], 1e-5)
        magic = cons.tile([128, 1], mybir.dt.int32)
        nc.vector.memset(magic[:], 0x5f3759df)
        # late consts (scalar queue, gated behind the last gather chunk)
        ident = cons.tile([128, 128], F32)
        gb_sb = cons.tile([128, 2, NCH], F32)
        wt_sb = cons.tile([128, 2, VL], BF16)
        logvm = cons.tile([128, NCH, B], F32)
        # emb mega-tile [128, t, h, m, r, s] (s=1 half pre-scaled by bvec)
        emb = embp.tile([128, 4, 2, M, 4, 2], FR, tag="emb", name="emb")
        # combined s0+s1 copy for the o-role: [128, t, h, m, r]
        embc = embp.tile([128, 4, 2, M, 4], F32, tag="embc", name="embc")

        gate_val = [0]

        sidx_use = sidx_sb
        prev_out = None
        for rep in range(repeat):
            if serial and rep:
                nc.all_engine_barrier()
            if chain and rep and prev_out is not None:
                # benchmark-only: serialize reps by deriving this rep's story
                # indices from the previous rep's output (forces true
                # per-iteration latency instead of pipelined throughput)
                z16 = tmp.tile([128, 1], I16, tag="z16")
                nc.vector.tensor_scalar_mul(z16[:], prev_out[:, 0, 0:1], 0.0)
                sidx_use = tmp.tile([128, M * 8], I16, tag="sidxg")
                nc.vector.tensor_tensor(
                    sidx_use[:], sidx_sb[:],
                    bass.AP(z16.tensor, z16[:].offset,
                            [z16[:].ap[0], [0, M * 8]]),
                    AluOpType.add)
            # wmatq = wmat * qm  (bf16)
            wmatq = tmp.tile([128, 8], BF16, tag="wmatq")
            nc.vector.tensor_scalar_mul(wmatq[:], wmat[:], qm_sb[:, 0:1])

            # ---- query encode ----
            rq = rt_p.tile([128, 1, 4 * E], BF16, tag="rq")
            nc.gpsimd.dma_gather(rq[:], tabc, qidx_sb[:, :],
                                 num_idxs=128, num_idxs_reg=128,
                                 elem_size=4 * E)
            ps_q = pp_t.tile([128, 16], F32, tag="aux")
            for h in range(2):
                nc.tensor.matmul(ps_q[:, h * 8:(h + 1) * 8],
                                 rq[:, 0, h * EH:(h + 1) * EH],
                                 wmatq[:], start=True, stop=True)
            q_sb = tmp.tile([128, 16], F32, tag="q_sb")
            nc.vector.tensor_copy(q_sb[:], ps_q[:])
            u_cur = up.tile([128, 2, BL], FR, tag="u")
            for h in range(2):
                psq_odd = bass.AP(q_sb.tensor, q_sb[:].offset + h * 8 + 1,
                                  [q_sb[:].ap[0], [2, BL]])
                psq_evn = bass.AP(q_sb.tensor, q_sb[:].offset + h * 8 + 0,
                                  [q_sb[:].ap[0], [2, BL]])
                nc.vector.scalar_tensor_tensor(
                    u_cur[:, h, :], psq_odd, bvec[:, h:h + 1], psq_evn,
                    AluOpType.mult, AluOpType.add)

            # ---- story gathers (4 tables fused) + PE reductions ----
            nchunks = (M + NQ - 1) // NQ
            if stage >= 2:
              for j in range(nchunks):
                    q0 = j * NQ
                    nq = min(NQ, M - q0)
                    rt = rt_p.tile([128, NQ, 4 * E], BF16, tag="rt")
                    nc.gpsimd.dma_gather(
                        rt[:, :nq, :], tabc,
                        sidx_use[:, q0 * 8:(q0 + nq) * 8],
                        num_idxs=nq * 128, num_idxs_reg=nq * 128,
                        elem_size=4 * E)
                    ps_e = pp_e.tile([128, NQ * 64], F32, tag="pse")
                    for q in range(nq):
                        for t in range(4):
                            for h in range(2):
                                nc.tensor.matmul(
                                    ps_e[:, q * 64 + t * 16 + h * 8:
                                         q * 64 + t * 16 + h * 8 + 8],
                                    rt[:, q, t * E + h * EH:
                                       t * E + (h + 1) * EH],
                                    wmat[:], start=True, stop=True)
                    # merged flush: s=0 plain copy; s=1 pre-scaled by
                    # bvec (folds the position-encoding b_e into emb so the
                    # hops need no separate u*bvec operand)
                    src0 = bass.AP(ps_e.tensor, ps_e[:].offset,
                                   [ps_e[:].ap[0], [64, nq], [16, 4],
                                    [8, 2], [2, 4]])
                    dst0 = bass.AP(emb.tensor, emb[:].offset + q0 * 8,
                                   [emb[:].ap[0], [8, nq], [1600, 4],
                                    [800, 2], [2, 4]])
                    nc.vector.tensor_copy(dst0, src0)
                    for h in range(2):
                        src1 = bass.AP(ps_e.tensor,
                                       ps_e[:].offset + h * 8 + 1,
                                       [ps_e[:].ap[0], [64, nq], [16, 4],
                                        [2, 4]])
                        dst1 = bass.AP(emb.tensor,
                                       emb[:].offset + q0 * 8 + h * 800 + 1,
                                       [emb[:].ap[0], [8, nq], [1600, 4],
                                        [2, 4]])
                        nc.vector.tensor_scalar_mul(dst1, src1,
                                                    bvec[:, h:h + 1])
                        e0 = bass.AP(emb.tensor,
                                     emb[:].offset + q0 * 8 + h * 800,
                                     [emb[:].ap[0], [8, nq], [1600, 4],
                                      [2, 4]])
                        e1 = bass.AP(emb.tensor,
                                     emb[:].offset + q0 * 8 + h * 800 + 1,
                                     [emb[:].ap[0], [8, nq], [1600, 4],
                                      [2, 4]])
                        dstc = bass.AP(embc.tensor,
                                       embc[:].offset + q0 * 4 + h * 400,
                                       [embc[:].ap[0], [4, nq], [800, 4],
                                        [1, 4]])
                        nc.vector.tensor_tensor(dstc, e0, e1, AluOpType.add)
              if rep == 0:
                # late const loads: force a WAW dep on the chunk-8 emb flush
                # so the 2.5MB of wt/logvm traffic stays out of the
                # gather-critical DMA window (the scheduler orders by deps,
                # not emission)
                nc.vector.tensor_copy(wt_sb[0:1, 0, 0:2],
                                      emb[0:1, 0, 0, 96, 0, :])
                nc.vector.tensor_copy(wt_sb[0:1, 1, 0:2],
                                      emb[0:1, 0, 0, 96, 0, :])
                nc.vector.tensor_copy(logvm[0:1, 0, 0:2],
                                      emb[0:1, 0, 0, 96, 0, :])
                nc.scalar.dma_start(ident[:], idi)
                nc.scalar.dma_start(gb_sb[:], gbi)
                nc.scalar.dma_start(wt_sb[:, 0, :], wti[0])
                nc.scalar.dma_start(wt_sb[:, 1, :], wti[1])
                nc.scalar.dma_start(logvm[:], vmi)

            # ---- hops ----
            for hop in range(nhop if stage >= 3 else 0):
                eA = emb[:].offset + hop * 1600
                eC = emb[:].offset + (hop + 1) * 1600
                # scores: 4 accumulating cross-product matmuls -> [4, 400]
                ps_sc = pp_s.tile([4, 400], F32, tag="scr")
                if dbg_terms:
                    ps_t4 = pp_b.tile([4, 512], F32, tag="t4")
                k = 0
                for h in range(2):
                    for s in range(2):
                        lhsT = u_cur[:, h, :]
                        rhs = bass.AP(emb.tensor, eA + h * 800 + s,
                                      [emb[:].ap[0], [2, 4], [8, M]])
                        nc.tensor.matmul(ps_sc[:], lhsT, rhs,
                                         start=(k == 0), stop=(k == 3))
                        if dbg_terms and k == dbg_terms - 1:
                            nc.tensor.matmul(ps_t4[:, 0:400],
                                             lhsT, rhs, start=True, stop=True)
                        k += 1
                # masked softmax on [4, 400] (valid = block diag; PM is a
                # 0/1 mask so the -1e30 add alone implements masking and the
                # exp's accumulator gives the row sum)
                sm = tmp.tile([4, 400], F32, tag="sm")
                nc.vector.tensor_add(sm[:], ps_sc[:], ma_sb[:])
                mx = tmp.tile([4, 1], F32, tag="mx")
                nc.vector.tensor_reduce(mx[:], sm[:], AX.X, AluOpType.max,
                                        negate=True)
                es = tmp.tile([4, 400], F32, tag="es4")
                ssum = tmp.tile([4, 1], F32, tag="ssum")
                nc.scalar.activation(es[:], sm[:], AF.Exp, bias=mx[:],
                                     scale=1.0, accum_out=ssum[:])
                nc.vector.tensor_mul(es[:], es[:], mm_sb[:])
                nc.vector.tensor_scalar_add(ssum[:], ssum[:], 1e-13)
                nc.vector.reciprocal(ssum[:], ssum[:])
                pw = tmp.tile([4, 400], FR, tag="pw")
                nc.vector.tensor_scalar_mul(pw[:], es[:], ssum[:, 0:1])
                # broadcast + block-diag extract: [128, 400]
                ps_p = pp_b.tile([128, 400], F32, tag="pb")
                nc.tensor.matmul(ps_p[:], on4[:], pw[:],
                                 start=True, stop=True)
                # o[(h,s,r)] = sum_m embC * p
                # o[(h,r)] = sum_m memC * p via the s-combined emb copy
                eCc = embc[:].offset + (hop + 1) * 800
                scr = big.tile([128, 2, 4, M], F32, tag="scr8")
                o_t = tmp.tile([128, 2, 4], F32, tag="ot")
                in0 = bass.AP(embc.tensor, eCc,
                              [embc[:].ap[0], [400, 2], [1, 4], [4, M]])
                in1 = bass.AP(ps_p.tensor, ps_p[:].offset,
                              [ps_p[:].ap[0], [0, 2], [100, 4], [1, M]])
                nc.vector.tensor_tensor(scr[:], in0, in1, AluOpType.mult)
                nc.vector.tensor_reduce(
                    o_t[:], bass.AP(scr.tensor, scr[:].offset,
                                    [scr[:].ap[0], [100, 8], [1, 100]]),
                    AX.X, AluOpType.add)
                u_nxt = up.tile([128, 2, BL], FR, tag="u")
                nc.vector.tensor_add(u_nxt[:], u_cur[:], o_t[:])
                u_cur = u_nxt

            # ---- AllGather u3 ----
            if stage < 4:
                out_sb0 = big.tile([128, NCH, B], F32, tag="outsb")
                nc.vector.memset(out_sb0[:], 0.0)
                nc.vector.tensor_copy(
                    out_sb0[:, 0, 0:8],
                    bass.AP(u_cur.tensor, u_cur[:].offset,
                            [u_cur[:].ap[0], [1, 8]]))
                if stage == 2:
                    # dump emb[t=0..3, h, m=0, r, s] -> cols [128, 192)
                    nc.vector.tensor_copy(
                        bass.AP(out_sb0.tensor, out_sb0[:].offset + 128,
                                [out_sb0[:].ap[0], [16, 4], [8, 2], [1, 8]]),
                        bass.AP(emb.tensor, emb[:].offset,
                                [emb[:].ap[0], [1600, 4], [800, 2], [1, 8]]))
                if dbg_terms and nhop >= 1:
                    nc.vector.tensor_copy(
                        bass.AP(out_sb0.tensor, out_sb0[:].offset + 600,
                                [[1024, 4], [1, 400]]),
                        ps_t4[:, 0:400])
                if stage >= 3 and nhop >= 1:
                    # debug: pw at cols [128, 528), sm at [600, 1000)
                    nc.vector.tensor_copy(
                        bass.AP(out_sb0.tensor, out_sb0[:].offset + 128,
                                [[1024, 4], [1, 400]]),
                        pw[:])

                nc.sync.dma_start(out, out_sb0[:])
                continue
            nc.sync.dma_start(
                bass.AP(u3_loc.tensor, 0,
                        [[BL, 128], [128 * BL, 2], [1, BL]]),
                u_cur[:].bitcast(F32))
            nc.gpsimd.collective_compute(
                "AllGather", AluOpType.bypass,
                replica_groups=[list(range(NC_))],
                ins=[u3_loc], outs=[u3_gth])
            for _x in range(xcoll):
                nc.gpsimd.collective_compute(
                    "AllGather", AluOpType.bypass,
                    replica_groups=[list(range(NC_))],
                    ins=[u3_loc], outs=[u3_gth])
            u3g = tmp.tile([128, 2, B], F32, tag="u3g")
            for h in range(2):
                nc.sync.dma_start(
                    u3g[:, h, :],
                    bass.AP(u3_gth.tensor, h * 128 * BL,
                            [[BL, 128], [2 * 128 * BL, NC_], [1, BL]]))
            u3b = tmp.tile([128, 2, B], BF16, tag="u3b")
            nc.vector.tensor_copy(u3b[:], u3g[:])

            # ---- final matmul: wx[v, b] ----
            ps_wx = pp_w.tile([128, NCH, B], F32, tag="pswx")
            for pb in (32, 64, 96):
                nc.vector.memset(ps_wx[pb:pb + 32, NCH - 1, :], 0.0)
            for ch in range(NCH):
                m_sz = min(CH, VL - ch * CH)
                for h in range(2):
                    nc.tensor.matmul(
                        ps_wx[:m_sz, ch, :],
                        wt_sb[:, h, ch * CH:ch * CH + m_sz],
                        u3b[:, h, :],
                        start=(h == 0), stop=(h == 1))

            # ---- BatchNorm stats (mean/var over batch, per vocab col) ----
            mean = tmp.tile([128, NCH], F32, tag="mean")
            nc.vector.tensor_reduce(mean[:], ps_wx[:], AX.X, AluOpType.add)
            nc.vector.tensor_scalar_mul(mean[:], mean[:], 1.0 / B)
            sq = big.tile([128, NCH, B], F32, tag="sq")
            nc.scalar.activation(sq[:], ps_wx[:], AF.Square)
            var = tmp.tile([128, NCH], F32, tag="var")
            nc.vector.tensor_reduce(var[:], sq[:], AX.X, AluOpType.add)
            nc.vector.tensor_scalar_mul(var[:], var[:], 1.0 / B)
            msq = tmp.tile([128, NCH], F32, tag="msq")
            nc.vector.tensor_mul(msq[:], mean[:], mean[:])
            nc.vector.tensor_sub(var[:], var[:], msq[:])
            # rstd = rsqrt(var+eps) via bit-trick + 2 Newton steps on DVE
            # (avoids the Sqrt act-table, whose load/unload costs ~2.6us)
            veps = tmp.tile([128, NCH], F32, tag="veps")
            nc.vector.tensor_scalar_add(veps[:], var[:], 1e-5)
            I32 = mybir.dt.int32
            rstd = tmp.tile([128, NCH], F32, tag="rstd")
            ri = rstd[:].bitcast(I32)
            nc.vector.tensor_scalar(ri, veps[:].bitcast(I32), 1, None,
                                    AluOpType.logical_shift_right)
            magic_b = bass.AP(magic.tensor, magic[:].offset,
                              [magic[:].ap[0], [0, NCH]])
            nc.vector.tensor_tensor(ri, magic_b, ri, AluOpType.subtract)
            half_v = tmp.tile([128, NCH], F32, tag="halfv")
            nc.vector.tensor_scalar_mul(half_v[:], veps[:], -0.5)
            nwt = tmp.tile([128, NCH], F32, tag="nwt")
            for _it in range(2):
                nc.vector.tensor_mul(nwt[:], rstd[:], rstd[:])
                nc.vector.tensor_mul(nwt[:], nwt[:], half_v[:])
                nc.vector.scalar_tensor_tensor(rstd[:], nwt[:], 1.5, rstd[:],
                                               AluOpType.add, AluOpType.mult)
            av = tmp.tile([128, NCH], F32, tag="av")
            nc.vector.tensor_mul(av[:], gb_sb[:, 0, :], rstd[:])
            bv = tmp.tile([128, NCH], F32, tag="bv")
            nc.vector.tensor_mul(bv[:], av[:], mean[:])
            nc.vector.tensor_sub(bv[:], gb_sb[:, 1, :], bv[:])
            # y = av*wx + (bv + logvm)
            lv2 = big.tile([128, NCH, B], F32, tag="lv2")
            bv_b = bass.AP(bv.tensor, bv[:].offset,
                           [bv[:].ap[0], [1, NCH], [0, B]])
            nc.gpsimd.tensor_tensor(lv2[:], logvm[:], bv_b, AluOpType.add)
            # y + exp + reduce pipelined in vocab-chunk halves so the Act
            # exp of one half overlaps the DVE work of the other
            y_all = big.tile([128, NCH, B], F32, tag="yall")
            es_h = big.tile([128, NCH, B], F32, tag="esh")
            xs2 = tmp.tile([128, 2, B], F32, tag="xs2")
            hn = NCH // 2
            for hf in range(2):
                sl = slice(hf * hn, (hf + 1) * hn)
                av_b = bass.AP(av.tensor, av[:].offset + hf * hn,
                               [av[:].ap[0], [1, hn], [0, B]])
                nc.vector.tensor_tensor(y_all[:, sl], ps_wx[:, sl], av_b,
                                        AluOpType.mult)
                nc.vector.tensor_add(y_all[:, sl], y_all[:, sl], lv2[:, sl])
                nc.scalar.activation(es_h[:, sl], y_all[:, sl], AF.Exp)
                nc.vector.tensor_reduce(
                    xs2[:, hf, :],
                    bass.AP(es_h.tensor, es_h[:].offset + hf * hn * B,
                            [es_h[:].ap[0], [1, B], [B, hn]]),
                    AX.X, AluOpType.add)
            xs = tmp.tile([128, B], F32, tag="xs")
            nc.vector.tensor_add(xs[:], xs2[:, 0, :], xs2[:, 1, :])
            ps_sl = pp_t.tile([B, 1], F32, tag="aux")
            nc.tensor.matmul(ps_sl[:], xs[:], onc[:], start=True, stop=True)
            s_loc = tmp.tile([B, 1], F32, tag="sloc")
            nc.vector.tensor_copy(s_loc[:], ps_sl[:])
            nc.sync.dma_start(lse_loc, s_loc[:, 0:1])
            nc.gpsimd.collective_compute(
                "AllGather", AluOpType.bypass,
                replica_groups=[list(range(NC_))],
                ins=[lse_loc], outs=[lse_gth])
            lse8 = tmp.tile([B, NC_], F32, tag="lse8")
            nc.sync.dma_start(lse8[:], bass.AP(lse_gth.tensor, 0,
                                               [[1, B], [B, NC_]]))
            s8 = tmp.tile([B, 1], F32, tag="s8")
            nc.vector.tensor_reduce(s8[:], lse8[:], AX.X, AluOpType.add)
            glse = tmp.tile([B, 1], F32, tag="glse")
            nc.scalar.activation(glse[:], s8[:], AF.Ln)
            # broadcast glse over partitions: [B,1] -> [1,B] -> K=1 matmul
            ps_g1 = pp_t.tile([1, B], F32, tag="aux")
            nc.tensor.transpose(ps_g1[:], glse[:], ident[:B, :B])
            g_row = tmp.tile([1, B], F32, tag="grow")
            nc.vector.tensor_copy(g_row[:], ps_g1[:])
            ps_gb = pp_t.tile([128, B], F32, tag="aux")
            nc.tensor.matmul(ps_gb[:], ones1[:], g_row[:], start=True, stop=True)
            out_sb = big.tile([128, NCH, B], F32, tag="outsb")
            hn = NCH // 2
            gb_h = bass.AP(ps_gb.tensor, ps_gb[:].offset,
                           [ps_gb[:].ap[0], [0, hn], [1, B]])
            for hf in range(2):
                nc.vector.tensor_tensor(out_sb[:, hf * hn:(hf + 1) * hn],
                                        y_all[:, hf * hn:(hf + 1) * hn],
                                        gb_h, AluOpType.subtract)
                nc.sync.dma_start(out[:, hf * hn:(hf + 1) * hn],
                                  out_sb[:, hf * hn:(hf + 1) * hn])
            prev_out = out_sb

    nc.compile()
    return nc


def marshal(inputs):
    """FULL inputs -> per-core in_maps."""
    wmat, bvec, ones1, on4, onc, ident = _consts()
    trainS = np.asarray(inputs['trainS'])
    trainQ = np.asarray(inputs['trainQ'])
    trainVM = np.asarray(inputs['trainVM'], dtype=np.float32)
    trainPM = np.asarray(inputs['trainPM'], dtype=np.float32)
    trainQM = np.asarray(inputs['trainQM'], dtype=np.float32)
    tabc = np.concatenate(
        [np.asarray(inputs[k], dtype=np.float32) for k in ('A1', 'A2', 'A3', 'A4')],
        axis=1).astype(ml_dtypes.bfloat16)
    W = np.asarray(inputs['W'], dtype=np.float32)
    gamma = np.asarray(inputs['gamma'], dtype=np.float32)
    beta = np.asarray(inputs['beta'], dtype=np.float32)

    in_maps = []
    for c in range(NC_):
        rb = slice(BL * c, BL * (c + 1))
        vs = VL * c
        arr = np.zeros((M, BL, 32), np.int16)
        arr[:, :, :L] = trainS[rb].transpose(1, 0, 2)
        sidx = _wrap_idx(arr.reshape(-1))
        qa = np.zeros((BL, 32), np.int16)
        qa[:, :LQ] = trainQ[rb, 0, :]
        qidx = _wrap_idx(qa.reshape(-1))
        qmc = np.zeros((128, 1), np.float32)
        for r in range(BL):
            qmc[32 * r:32 * r + LQ, 0] = trainQM[BL * c + r]
        # score masks [4, 400]: col j = rp*100 + m
        pm = trainPM[rb]                                   # [4, 100]
        mm = np.zeros((4, 4, 100), np.float32)
        ma = np.full((4, 4, 100), -1e30, np.float32)
        for r in range(4):
            mm[r, r] = pm[r]
            ma[r, r] = np.where(pm[r] > 0, 0.0, -1e30)
        wt = W[vs:vs + VL].T.reshape(2, 128, VL).astype(ml_dtypes.bfloat16)
        gb = np.zeros((128, 2, NCH), np.float32)
        gpad = np.zeros(NCH * CH, np.float32); gpad[:VL] = gamma[vs:vs + VL]
        bpad = np.zeros(NCH * CH, np.float32); bpad[:VL] = beta[vs:vs + VL]
        gb[:, 0, :] = gpad.reshape(NCH, CH).T
        gb[:, 1, :] = bpad.reshape(NCH, CH).T
        # log(VM + 1e-13), pad vocab rows forced to -1e30 so their exp is 0
        vmt = np.zeros((128, NCH, B), np.float32)
        lv = np.log(trainVM[:, vs:vs + VL].astype(np.float64)
                    + 1e-13).astype(np.float32).T          # [VL, B]
        lvp = np.full((NCH * CH, B), -1e30, np.float32); lvp[:VL] = lv
        vmt[:] = lvp.reshape(NCH, CH, B).transpose(1, 0, 2)
        in_maps.append({
            'tabc': tabc,
            'sidx': sidx, 'qidx': qidx, 'qmc': qmc,
            'mmi': mm.reshape(4, 400), 'mai': ma.reshape(4, 400),
            'wti': wt, 'gbi': gb, 'vmi': vmt,
            'wmi': wmat.astype(ml_dtypes.bfloat16), 'bvi': bvec,
            'on1': ones1, 'on4i': on4, 'onci': onc, 'idi': ident,
        })
    return in_maps


def unmarshal(results):
    outf = np.zeros((B, V), np.float32)
    for c in range(NC_):
        o = np.asarray(results[c]['out']).reshape(128, NCH, B)
        outf[:, VL * c:VL * (c + 1)] = \
            o.transpose(2, 1, 0).reshape(B, NCH * CH)[:, :VL]
    return outf


def kernel(**inputs):
    if 'nc' not in _cache:
        _cache['nc'] = build_nc()
    nc = _cache['nc']
    in_maps = marshal(inputs)
    res = run_bass_kernel_spmd(nc, in_maps, list(range(NC_)))
    return unmarshal(res.results)
